# revision 2
# baseline (speedup 1.0000x reference)
"""AspectAttention Trainium2 kernel (8 NeuronCores, pure data parallel).

out[b, n] = sum_e softmax_n(tanh(h @ W_a + b_a))[b, n, e] * h[b, n, e]

Self-contained: hardcodes shapes B=4096, N=64, D=256, 8 cores.

Per-core dataflow (512 batches), batch-on-partitions layout:
  - SWDGE cast-DMA: h f32 DRAM -> bf16 SBUF chunks [128 b, 64 n, 256 d]
    (64KB contiguous per partition -> full-rate descriptors)
  - PE transpose per (n, d-half): [128 b, 128 d] -> PSUM [128 d, 128 b] (bf16)
  - DVE copies PSUM -> SBUF hT slabs
  - PE matmul: stationary hT-slab [d, b], moving W [d, e] -> PSUM s[b, e],
    accumulated over both d-halves
  - ScalarE: tanh (PSUM->SBUF bf16), then exp. tanh in [-1,1] bounds the
    softmax domain, so no max-subtraction is needed.
  - DVE tree-sum over n -> S[b, e]; reciprocal -> R
  - GPSIMD: h' = h * R (R broadcast over n via 0-stride AP)
  - DVE tensor_tensor_reduce: out[:, n] = sum_e E[:, n, :] * h'[:, n, :]
"""
import math
from contextlib import ExitStack

import numpy as np

import concourse.bass as bass
import concourse.tile as tile
from concourse import bacc, mybir
from concourse.bass_utils import run_bass_kernel_spmd

N_CORES = 8
B_FULL, N_BLOCK, D = 4096, 64, 256
B_SHARD = B_FULL // N_CORES  # 512
P = 128
N_CHUNKS = B_SHARD // P  # 4
F32 = mybir.dt.float32
BF16 = mybir.dt.bfloat16
ALU = mybir.AluOpType
ACT_T = mybir.ActivationFunctionType


def build_nc(with_bias: bool):
    nc = bacc.Bacc("TRN2", debug=False, num_devices=N_CORES)
    h_d = nc.dram_tensor("h", [B_SHARD, N_BLOCK, D], F32, kind="ExternalInput")
    w_d = nc.dram_tensor("W_a", [D, D], F32, kind="ExternalInput")
    ident_d = nc.dram_tensor("ident", [P, P], BF16, kind="ExternalInput")
    if with_bias:
        ba_d = nc.dram_tensor("b_a", [N_BLOCK, D], F32, kind="ExternalInput")
    out_d = nc.dram_tensor("out", [B_SHARD, N_BLOCK], F32, kind="ExternalOutput")

    with tile.TileContext(nc) as tc, ExitStack() as ctx:
        const_p = ctx.enter_context(tc.tile_pool(name="const", bufs=1))
        h_p = ctx.enter_context(tc.tile_pool(name="h", bufs=2))
        e_p = ctx.enter_context(tc.tile_pool(name="E", bufs=2))
        ht_p = ctx.enter_context(tc.tile_pool(name="hT", bufs=2))
        t_p = ctx.enter_context(tc.tile_pool(name="t", bufs=2))
        hp_p = ctx.enter_context(tc.tile_pool(name="hp", bufs=2))
        tr_p = ctx.enter_context(tc.tile_pool(name="tr", bufs=1))
        s_p = ctx.enter_context(tc.tile_pool(name="S", bufs=2))
        scr_p = ctx.enter_context(tc.tile_pool(name="scr", bufs=4))
        o_p = ctx.enter_context(tc.tile_pool(name="o", bufs=2))
        psT_p = ctx.enter_context(tc.tile_pool(name="psT", bufs=2, space="PSUM"))
        psS_p = ctx.enter_context(tc.tile_pool(name="psS", bufs=2, space="PSUM"))

        # constants: identity (for PE transpose), W in bf16
        ident = const_p.tile([P, P], BF16)
        nc.sync.dma_start(ident[:], ident_d.ap())
        wf = const_p.tile([P, 2, D], F32)
        nc.sync.dma_start(wf[:, 0, :], w_d.ap()[0:P, :])
        nc.sync.dma_start(wf[:, 1, :], w_d.ap()[P : 2 * P, :])
        wb = const_p.tile([P, 2, D], BF16)
        nc.vector.tensor_copy(wb[:], wf[:])
        if with_bias:
            # broadcast b_a across all 128 partitions (DRAM 0-stride read)
            bab = const_p.tile([P, N_BLOCK, D], BF16)
            src = ba_d.ap().rearrange("(one n) d -> one n d", one=1)
            src = src.broadcast_to((P, N_BLOCK, D))
            nc.gpsimd.dma_start(bab[:], src)

        for c in range(N_CHUNKS):
            bs = c * P
            h_nat = h_p.tile([P, N_BLOCK, D], BF16)
            for g in range(4):  # 4 sub-loads of 16 n each (2MB reads)
                nc.gpsimd.dma_start(
                    h_nat[:, g * 16 : (g + 1) * 16, :],
                    h_d.ap()[bs : bs + P, g * 16 : (g + 1) * 16, :],
                )
            E = e_p.tile([P, N_BLOCK, D], BF16)
            for ng in range(8):  # n-groups of 8
                psT = psT_p.tile([P, 16, P], BF16)
                for j in range(8):
                    n = ng * 8 + j
                    for dh in range(2):
                        nc.tensor.transpose(
                            psT[:, 2 * j + dh, :],
                            h_nat[:, n, dh * P : (dh + 1) * P],
                            ident[:],
                        )
                hT = ht_p.tile([P, 16, P], BF16)
                nc.vector.tensor_copy(hT[:], psT[:])
                for q in range(2):  # 4-n sub-groups for PSUM/ACT tiles
                    psS = psS_p.tile([P, 4, D], F32)
                    for j in range(4):
                        jj = q * 4 + j
                        n = ng * 8 + jj
                        nc.tensor.matmul(
                            psS[:, j, :], hT[:, 2 * jj, :], wb[:, 0, :],
                            start=True, stop=False,
                        )
                        nc.tensor.matmul(
                            psS[:, j, :], hT[:, 2 * jj + 1, :], wb[:, 1, :],
                            start=False, stop=True,
                        )
                    n0 = ng * 8 + q * 4
                    t = t_p.tile([P, 4, D], BF16)
                    if with_bias:
                        tb = t_p.tile([P, 4, D], F32, tag="tbias")
                        nc.vector.tensor_add(tb[:], psS[:], bab[:, n0 : n0 + 4, :])
                        nc.scalar.activation(t[:], tb[:], ACT_T.Tanh)
                    else:
                        nc.scalar.activation(t[:], psS[:], ACT_T.Tanh)
                    nc.scalar.activation(E[:, n0 : n0 + 4, :], t[:], ACT_T.Exp)

            # tree-sum over n -> S [128, 256] f32
            tr1 = tr_p.tile([P, 32, D], BF16, tag="tr1")
            tr2 = tr_p.tile([P, 16, D], BF16, tag="tr2")
            nc.gpsimd.tensor_tensor(tr1[:], E[:, 0:32, :], E[:, 32:64, :], ALU.add)
            nc.vector.tensor_tensor(tr2[:], tr1[:, 0:16, :], tr1[:, 16:32, :], ALU.add)
            nc.vector.tensor_tensor(tr1[:, 0:8, :], tr2[:, 0:8, :], tr2[:, 8:16, :], ALU.add)
            nc.vector.tensor_tensor(tr2[:, 0:4, :], tr1[:, 0:4, :], tr1[:, 4:8, :], ALU.add)
            nc.vector.tensor_tensor(tr1[:, 0:2, :], tr2[:, 0:2, :], tr2[:, 2:4, :], ALU.add)
            S = s_p.tile([P, D], F32, tag="S")
            nc.vector.tensor_tensor(S[:], tr1[:, 0, :], tr1[:, 1, :], ALU.add)
            R = s_p.tile([P, D], F32, tag="R")
            nc.vector.reciprocal(R[:], S[:])
            Rb = s_p.tile([P, D], BF16, tag="Rb")
            nc.vector.tensor_copy(Rb[:], R[:])
            Rb_b = Rb[:, :].rearrange("p (one e) -> p one e", one=1).broadcast_to(
                (P, 16, D)
            )

            out_sb = o_p.tile([P, N_BLOCK], F32, tag="out_sb")
            for g in range(4):  # 16-n groups
                hp = hp_p.tile([P, 16, D], BF16)
                nc.gpsimd.tensor_tensor(
                    hp[:], h_nat[:, g * 16 : (g + 1) * 16, :], Rb_b, ALU.mult
                )
                for j in range(16):
                    n = g * 16 + j
                    scr = scr_p.tile([P, D], BF16)
                    nc.vector.scalar_tensor_tensor(
                        out=scr[:],
                        in0=E[:, n, :],
                        scalar=1.0,
                        in1=hp[:, j, :],
                        op0=ALU.mult,
                        op1=ALU.mult,
                        accum_out=out_sb[:, n : n + 1],
                    )
            staged = o_p.tile([P, N_BLOCK], F32, tag="staged")
            nc.vector.tensor_copy(staged[:], out_sb[:])
            nc.gpsimd.dma_start(out_d.ap()[bs : bs + P, :], staged[:])
    nc.compile()
    return nc


_CACHE = {}


def _get_nc(with_bias: bool):
    if with_bias not in _CACHE:
        _CACHE[with_bias] = build_nc(with_bias)
    return _CACHE[with_bias]


def run(h, W_a, b_a, trace=False):
    import ml_dtypes

    h = np.ascontiguousarray(np.asarray(h, dtype=np.float32))
    W_a = np.ascontiguousarray(np.asarray(W_a, dtype=np.float32))
    b_a = np.ascontiguousarray(np.asarray(b_a, dtype=np.float32))
    with_bias = bool(np.any(b_a))
    nc = _get_nc(with_bias)
    ident = np.eye(P, dtype=ml_dtypes.bfloat16)
    in_maps = []
    for i in range(N_CORES):
        m = {
            "h": h[i * B_SHARD : (i + 1) * B_SHARD],
            "W_a": W_a,
            "ident": ident,
        }
        if with_bias:
            m["b_a"] = b_a
        in_maps.append(m)
    res = run_bass_kernel_spmd(nc, in_maps, core_ids=list(range(N_CORES)), trace=trace)
    out = np.concatenate([res.results[i]["out"] for i in range(N_CORES)], axis=0)
    return out, res


def kernel(h, W_a, b_a):
    out, _ = run(h, W_a, b_a, trace=False)
    return out


# revision 3
# speedup vs baseline: 1.7841x; 1.7841x over previous
"""AspectAttention Trainium2 kernel (8 NeuronCores, pure data parallel).

out[b, n] = sum_e softmax_n(tanh(h @ W_a + b_a))[b, n, e] * h[b, n, e]

Self-contained: hardcodes shapes B=4096, N=64, D=256, 8 cores.

Per-core dataflow (512 batches), batch-on-partitions layout:
  - SWDGE cast-DMA: h f32 DRAM -> bf16 SBUF chunks [128 b, 64 n, 256 d]
    (64KB contiguous per partition -> full-rate descriptors)
  - PE transpose per (n, d-half): [128 b, 128 d] -> PSUM [128 d, 128 b] (bf16)
  - DVE copies PSUM -> SBUF hT slabs
  - PE matmul: stationary hT-slab [d, b], moving W [d, e] -> PSUM s[b, e],
    accumulated over both d-halves
  - ScalarE: tanh (PSUM->SBUF bf16), then exp. tanh in [-1,1] bounds the
    softmax domain, so no max-subtraction is needed.
  - DVE tree-sum over n -> S[b, e]; reciprocal -> R
  - GPSIMD: h' = h * R (R broadcast over n via 0-stride AP)
  - DVE tensor_tensor_reduce: out[:, n] = sum_e E[:, n, :] * h'[:, n, :]
"""
import math
from contextlib import ExitStack

import numpy as np

import concourse.bass as bass
import concourse.tile as tile
from concourse import bacc, mybir
from concourse.bass_utils import run_bass_kernel_spmd

N_CORES = 8
B_FULL, N_BLOCK, D = 4096, 64, 256
B_SHARD = B_FULL // N_CORES  # 512
P = 128
N_CHUNKS = B_SHARD // P  # 4
F32 = mybir.dt.float32
BF16 = mybir.dt.bfloat16
ALU = mybir.AluOpType
ACT_T = mybir.ActivationFunctionType


def build_nc(with_bias: bool):
    nc = bacc.Bacc("TRN2", debug=False, num_devices=N_CORES)
    h_d = nc.dram_tensor("h", [B_SHARD, N_BLOCK, D], F32, kind="ExternalInput")
    w_d = nc.dram_tensor("W_a", [D, D], F32, kind="ExternalInput")
    ident_d = nc.dram_tensor("ident", [P, P], BF16, kind="ExternalInput")
    if with_bias:
        ba_d = nc.dram_tensor("b_a", [N_BLOCK, D], F32, kind="ExternalInput")
    out_d = nc.dram_tensor("out", [B_SHARD, N_BLOCK], F32, kind="ExternalOutput")

    with tile.TileContext(nc) as tc, ExitStack() as ctx:
        const_p = ctx.enter_context(tc.tile_pool(name="const", bufs=1))
        h_p = ctx.enter_context(tc.tile_pool(name="h", bufs=2))
        e_p = ctx.enter_context(tc.tile_pool(name="E", bufs=2))
        ht_p = ctx.enter_context(tc.tile_pool(name="hT", bufs=2))
        t_p = ctx.enter_context(tc.tile_pool(name="t", bufs=2))
        hp_p = ctx.enter_context(tc.tile_pool(name="hp", bufs=2))
        tr_p = ctx.enter_context(tc.tile_pool(name="tr", bufs=1))
        s_p = ctx.enter_context(tc.tile_pool(name="S", bufs=2))
        scr_p = ctx.enter_context(tc.tile_pool(name="scr", bufs=4))
        o_p = ctx.enter_context(tc.tile_pool(name="o", bufs=2))
        psT_p = ctx.enter_context(tc.tile_pool(name="psT", bufs=2, space="PSUM"))
        psS_p = ctx.enter_context(tc.tile_pool(name="psS", bufs=2, space="PSUM"))

        # constants: identity (for PE transpose), W in bf16
        ident = const_p.tile([P, P], BF16)
        nc.sync.dma_start(ident[:], ident_d.ap())
        wf = const_p.tile([P, 2, D], F32)
        nc.sync.dma_start(wf[:, 0, :], w_d.ap()[0:P, :])
        nc.sync.dma_start(wf[:, 1, :], w_d.ap()[P : 2 * P, :])
        wb = const_p.tile([P, 2, D], BF16)
        nc.vector.tensor_copy(wb[:], wf[:])
        if with_bias:
            # broadcast b_a across all 128 partitions (DRAM 0-stride read)
            bab = const_p.tile([P, N_BLOCK, D], BF16)
            src = ba_d.ap().rearrange("(one n) d -> one n d", one=1)
            src = src.broadcast_to((P, N_BLOCK, D))
            nc.gpsimd.dma_start(bab[:], src)

        for c in range(N_CHUNKS):
            bs = c * P
            h_nat = h_p.tile([P, N_BLOCK, D], BF16)
            for g in range(4):  # 4 sub-loads of 16 n each (2MB reads)
                nc.gpsimd.dma_start(
                    h_nat[:, g * 16 : (g + 1) * 16, :],
                    h_d.ap()[bs : bs + P, g * 16 : (g + 1) * 16, :],
                )
            E = e_p.tile([P, N_BLOCK, D], BF16)
            for ng in range(8):  # n-groups of 8
                psT = psT_p.tile([P, 16, P], BF16)
                for j in range(8):
                    n = ng * 8 + j
                    for dh in range(2):
                        nc.tensor.transpose(
                            psT[:, 2 * j + dh, :],
                            h_nat[:, n, dh * P : (dh + 1) * P],
                            ident[:],
                        )
                hT = ht_p.tile([P, 16, P], BF16)
                nc.vector.tensor_copy(hT[:], psT[:])
                for q in range(2):  # 4-n sub-groups for PSUM/ACT tiles
                    psS = psS_p.tile([P, 4, D], F32)
                    for j in range(4):
                        jj = q * 4 + j
                        n = ng * 8 + jj
                        nc.tensor.matmul(
                            psS[:, j, :], hT[:, 2 * jj, :], wb[:, 0, :],
                            start=True, stop=False,
                        )
                        nc.tensor.matmul(
                            psS[:, j, :], hT[:, 2 * jj + 1, :], wb[:, 1, :],
                            start=False, stop=True,
                        )
                    n0 = ng * 8 + q * 4
                    t = t_p.tile([P, 4, D], BF16)
                    if with_bias:
                        tb = t_p.tile([P, 4, D], F32, tag="tbias")
                        nc.vector.tensor_add(tb[:], psS[:], bab[:, n0 : n0 + 4, :])
                        nc.scalar.activation(t[:], tb[:], ACT_T.Tanh)
                    else:
                        nc.scalar.activation(t[:], psS[:], ACT_T.Tanh)
                    nc.scalar.activation(E[:, n0 : n0 + 4, :], t[:], ACT_T.Exp)

            # tree-sum over n -> S [128, 256] f32
            tr1 = tr_p.tile([P, 32, D], BF16, tag="tr1")
            tr2 = tr_p.tile([P, 16, D], BF16, tag="tr2")
            nc.vector.tensor_tensor(tr1[:], E[:, 0:32, :], E[:, 32:64, :], ALU.add)
            nc.vector.tensor_tensor(tr2[:], tr1[:, 0:16, :], tr1[:, 16:32, :], ALU.add)
            nc.vector.tensor_tensor(tr1[:, 0:8, :], tr2[:, 0:8, :], tr2[:, 8:16, :], ALU.add)
            nc.vector.tensor_tensor(tr2[:, 0:4, :], tr1[:, 0:4, :], tr1[:, 4:8, :], ALU.add)
            nc.vector.tensor_tensor(tr1[:, 0:2, :], tr2[:, 0:2, :], tr2[:, 2:4, :], ALU.add)
            S = s_p.tile([P, D], F32, tag="S")
            nc.vector.tensor_tensor(S[:], tr1[:, 0, :], tr1[:, 1, :], ALU.add)
            R = s_p.tile([P, D], F32, tag="R")
            nc.vector.reciprocal(R[:], S[:])
            Rb = s_p.tile([P, D], BF16, tag="Rb")
            nc.vector.tensor_copy(Rb[:], R[:])

            out_sb = o_p.tile([P, N_BLOCK], F32, tag="out_sb")
            for g in range(4):  # 16-n groups
                hp = hp_p.tile([P, 16, D], BF16)
                nc.vector.tensor_tensor(
                    hp[:], E[:, g * 16 : (g + 1) * 16, :],
                    h_nat[:, g * 16 : (g + 1) * 16, :], ALU.mult
                )
                for j in range(16):
                    n = g * 16 + j
                    scr = scr_p.tile([P, D], BF16)
                    nc.vector.scalar_tensor_tensor(
                        out=scr[:],
                        in0=hp[:, j, :],
                        scalar=1.0,
                        in1=Rb[:],
                        op0=ALU.mult,
                        op1=ALU.mult,
                        accum_out=out_sb[:, n : n + 1],
                    )
            staged = o_p.tile([P, N_BLOCK], F32, tag="staged")
            nc.vector.tensor_copy(staged[:], out_sb[:])
            nc.gpsimd.dma_start(out_d.ap()[bs : bs + P, :], staged[:])
    nc.compile()
    return nc


_CACHE = {}


def _get_nc(with_bias: bool):
    if with_bias not in _CACHE:
        _CACHE[with_bias] = build_nc(with_bias)
    return _CACHE[with_bias]


def run(h, W_a, b_a, trace=False):
    import ml_dtypes

    h = np.ascontiguousarray(np.asarray(h, dtype=np.float32))
    W_a = np.ascontiguousarray(np.asarray(W_a, dtype=np.float32))
    b_a = np.ascontiguousarray(np.asarray(b_a, dtype=np.float32))
    with_bias = bool(np.any(b_a))
    nc = _get_nc(with_bias)
    ident = np.eye(P, dtype=ml_dtypes.bfloat16)
    in_maps = []
    for i in range(N_CORES):
        m = {
            "h": h[i * B_SHARD : (i + 1) * B_SHARD],
            "W_a": W_a,
            "ident": ident,
        }
        if with_bias:
            m["b_a"] = b_a
        in_maps.append(m)
    res = run_bass_kernel_spmd(nc, in_maps, core_ids=list(range(N_CORES)), trace=trace)
    out = np.concatenate([res.results[i]["out"] for i in range(N_CORES)], axis=0)
    return out, res


def kernel(h, W_a, b_a):
    out, _ = run(h, W_a, b_a, trace=False)
    return out
